# revision 48
# baseline (speedup 1.0000x reference)
"""GATv2 (2-layer, N=50000, E=800000) on 8 Trainium2 NeuronCores.

v2 strategy (self-contained; shapes hardcoded for nn_GATUnit_34067680592302):
  - Nodes partitioned across 8 cores (6250 each, 49 blocks of 128). Edges
    (incl. self-loops) assigned by destination block; within a block, edges
    with src < 32768 (padded id) come first, then src >= 32768, so layer-2
    table gathers can use int16-indexed dma_gather on two table halves.
  - Layer 1 needs NO gathers: the host streams per-edge transposed features
    x^T[:, src_e] and x^T[:, dst_e] (bf16), and e = x_src@Wl1 + x_dst@Wr1 is
    accumulated per 128-edge tile directly in PSUM by two matmuls.
  - Scores stay in edge-major layout: s = reduce_sum(prelu(e) * att_bc) on
    the vector engine; exp on scalar; weighted values w = xl_e * exp(s).
  - Scatter-softmax per block via mask matmuls: m[e, node] = (dst_e == node)
    accumulates [exp | w] into PSUM; epilogue divides by the denominators.
  - Layer 2: xl2 = h@Wl2 table is AllGather'ed (bf16) and row-gathered per
    block with two dma_gathers (lo/hi half).  xr2[dst_e] is reconstructed
    without a gather: dstrep = DMA partition-broadcast of the block's dst
    row, m2[node, e] = (dstrep == iota_col), and one matmul per tile expands
    the block-resident xr2 tile onto edges.
"""
import sys
sys.path.insert(0, "/opt/trn_rl_repo")

import numpy as np
import ml_dtypes

NEG = 0.2
BF = ml_dtypes.bfloat16


class Cfg:
    def __init__(self, N=50000, E=800000, ncores=8, nloc=6250, F=128):
        assert N == ncores * nloc
        self.N, self.E, self.ncores, self.nloc, self.F = N, E, ncores, nloc, F
        self.nblk = -(-nloc // 128)          # blocks of 128 nodes per core
        self.nlocp = self.nblk * 128         # padded local nodes
        self.npad = ncores * self.nlocp     # padded global nodes
        self.split = min(32768, self.npad)  # gather-table split (int16)
        assert self.npad % 512 == 0


CFG = Cfg()


def _r128(x):
    return int(-(-x // 128) * 128)


# --------------------------------------------------------------------------
# Host-side preprocessing
# --------------------------------------------------------------------------

def host_prep(x, edge_index, att1, b1, att2, b2, Wl1, Wr1, cfg):
    N, E, NC, NLOC = cfg.N, cfg.E, cfg.ncores, cfg.nloc
    NBLK, NLOCP, NPAD, F = cfg.nblk, cfg.nlocp, cfg.npad, cfg.F
    SPLIT = cfg.split

    src0 = np.asarray(edge_index[0]).astype(np.int64)
    dst0 = np.asarray(edge_index[1]).astype(np.int64)
    loops = np.arange(N, dtype=np.int64)
    SRC = np.concatenate([src0, loops])
    DST = np.concatenate([dst0, loops])
    shard = DST // NLOC
    src_g = ((SRC // NLOC) * NLOCP + (SRC % NLOC)).astype(np.int64)
    dst_loc = (DST - shard * NLOC).astype(np.int64)

    # per (core, block): lo/hi section counts
    per_core = []
    nlo = np.zeros((NC, NBLK), dtype=np.int64)
    nhi = np.zeros((NC, NBLK), dtype=np.int64)
    for c in range(NC):
        sel = np.nonzero(shard == c)[0]
        sg, so, do_, dl = src_g[sel], SRC[sel], DST[sel], dst_loc[sel]
        blk = dl // 128
        islo = sg < SPLIT
        # order: block-major, lo-then-hi within block (stable)
        order = np.lexsort((~islo, blk))
        sg, so, do_, dl, blk, islo = (a[order] for a in
                                      (sg, so, do_, dl, blk, islo))
        for b in range(NBLK):
            m = blk == b
            nlo[c, b] = int((islo & m).sum())
            nhi[c, b] = int((~islo & m).sum())
        per_core.append((sg, so, do_, dl, blk, islo))

    NLO_b = np.array([_r128(nlo[:, b].max()) for b in range(NBLK)])
    NHI_b = np.array([_r128(nhi[:, b].max()) for b in range(NBLK)])
    NTB_b = NLO_b + NHI_b
    offs = np.concatenate([[0], np.cumsum(NTB_b)]).astype(np.int64)
    NTOT = int(offs[-1])
    NT = NTOT // 128
    tiles_b = (NTB_b // 128).astype(np.int64)
    PCMAX = int(NTB_b.max())
    IDXCOLS = NTOT // 16

    H1 = att1.shape[0]
    C1 = att1.shape[1]
    att1bc = np.tile(np.asarray(att1, np.float32).reshape(1, H1 * C1),
                     (128, PCMAX // 128)).astype(BF)
    att2bc = np.tile(np.asarray(att2, np.float32).reshape(1, -1),
                     (128, PCMAX // 128)).astype(BF)

    iota = np.arange(128, dtype=np.float32)
    shared = dict(
        att1bc=att1bc, att2bc=att2bc,
        bias1r=np.tile(np.asarray(b1, np.float32), (128, 1)),
        bias2r=np.tile(np.asarray(b2, np.float32), (128, 1)),
        id128=np.eye(128, dtype=np.float32),
        iorowb=np.tile(iota, (128, 1)).astype(BF),
        iocolb=iota.reshape(128, 1).astype(BF),
        padmat=(np.arange(NLOCP) >= NLOC).astype(np.float32).reshape(1, NLOCP),
        e01=np.concatenate([np.ones(H1, np.float32),
                            np.zeros(F, np.float32)]).reshape(1, H1 + F),
        e02=np.concatenate([np.ones(1, np.float32),
                            np.zeros(F, np.float32)]).reshape(1, 1 + F),
        Wl1b=np.asarray(Wl1, np.float32).astype(BF),
        Wr1b=np.asarray(Wr1, np.float32).astype(BF),
    )

    x = np.asarray(x, dtype=np.float32)
    xbf = x.astype(BF)

    in_maps = []
    for c in range(NC):
        sg, so, do_, dl, blk, islo = per_core[c]
        cblo = np.concatenate([[0], np.cumsum(nlo[c])])
        cbhi = np.concatenate([[0], np.cumsum(nhi[c])])
        srco_arr = np.zeros(NTOT, dtype=np.int64)
        dsto_arr = np.zeros(NTOT, dtype=np.int64)
        dstb_arr = np.full(NTOT, -1.0, dtype=np.float32)
        idx16 = np.zeros(NTOT, dtype=np.int16)
        for b in range(NBLK):
            o = int(offs[b])
            ml = blk == b
            lo_i = np.nonzero(ml & islo)[0]
            hi_i = np.nonzero(ml & ~islo)[0]
            nl, nh = len(lo_i), len(hi_i)
            srco_arr[o:o + nl] = so[lo_i]
            dsto_arr[o:o + nl] = do_[lo_i]
            dstb_arr[o:o + nl] = dl[lo_i] - 128 * b
            idx16[o:o + nl] = sg[lo_i].astype(np.int16)
            o2 = o + int(NLO_b[b])
            srco_arr[o2:o2 + nh] = so[hi_i]
            dsto_arr[o2:o2 + nh] = do_[hi_i]
            dstb_arr[o2:o2 + nh] = dl[hi_i] - 128 * b
            idx16[o2:o2 + nh] = (sg[hi_i] - SPLIT).astype(np.int16)

        dstcol = np.ascontiguousarray(
            dstb_arr.reshape(NT, 128).T).astype(BF)
        dstrow = np.full((NBLK, PCMAX), -1.0, dtype=np.float32)
        for b in range(NBLK):
            dstrow[b, :int(NTB_b[b])] = \
                dstb_arr[int(offs[b]):int(offs[b + 1])]
        dstrow = dstrow.astype(BF)
        idxw = np.ascontiguousarray(idx16.reshape(IDXCOLS, 16).T)
        idxw = np.tile(idxw, (8, 1))

        xeT = np.ascontiguousarray(xbf[srco_arr].T)   # [128, NTOT]
        xdT = np.ascontiguousarray(xbf[dsto_arr].T)   # [128, NTOT]
        xloc = np.zeros((NLOCP, F), dtype=BF)
        xloc[:NLOC] = xbf[c * NLOC:(c + 1) * NLOC]
        xTloc = np.ascontiguousarray(xloc.T)          # [128, NLOCP]

        m = dict(shared)
        m.update(dstcol=dstcol, dstrow=dstrow, idx16=idxw, xeT=xeT, xdT=xdT,
                 xTloc=xTloc)
        in_maps.append(m)

    meta = dict(NLO_b=NLO_b.tolist(), NHI_b=NHI_b.tolist(),
                tiles_b=tiles_b.tolist(), offs=offs.tolist(), NT=NT,
                NTOT=NTOT, PCMAX=PCMAX, IDXCOLS=IDXCOLS, H1=H1)
    return in_maps, meta


# --------------------------------------------------------------------------
# Device program
# --------------------------------------------------------------------------

def build_nc(cfg, meta, use_prelu=True):
    import concourse.bacc as bacc
    import concourse.tile as tile
    from concourse import mybir

    f32 = mybir.dt.float32
    bf16 = mybir.dt.bfloat16
    i16 = mybir.dt.int16
    AF = mybir.ActivationFunctionType
    OP = mybir.AluOpType
    AX = mybir.AxisListType

    NC, F = cfg.ncores, cfg.F
    NBLK, NLOCP, NPAD = cfg.nblk, cfg.nlocp, cfg.npad
    SPLIT = cfg.split
    NT, NTOT, PCMAX = meta["NT"], meta["NTOT"], meta["PCMAX"]
    IDXCOLS, H1 = meta["IDXCOLS"], meta["H1"]
    NLO_b, NHI_b = meta["NLO_b"], meta["NHI_b"]
    tiles_b, offs = meta["tiles_b"], meta["offs"]

    nc = bacc.Bacc("TRN2", target_bir_lowering=False)

    din = {}
    def ein(name, shape, dt=f32):
        din[name] = nc.dram_tensor(name, shape, dt, kind="ExternalInput")
        return din[name]

    d_xeT = ein("xeT", [128, NTOT], bf16)
    d_xdT = ein("xdT", [128, NTOT], bf16)
    d_xTloc = ein("xTloc", [128, NLOCP], bf16)
    d_Wl1b, d_Wr1b = ein("Wl1b", [128, 128], bf16), ein("Wr1b", [128, 128], bf16)
    d_Wl2, d_Wr2 = ein("Wl2", [128, 128]), ein("Wr2", [128, 128])
    d_att1 = ein("att1bc", [128, PCMAX], bf16)
    d_att2 = ein("att2bc", [128, PCMAX], bf16)
    d_b1r, d_b2r = ein("bias1r", [128, F]), ein("bias2r", [128, F])
    d_id = ein("id128", [128, 128])
    d_iorowb = ein("iorowb", [128, 128], bf16)
    d_iocolb = ein("iocolb", [128, 1], bf16)
    d_padm = ein("padmat", [1, NLOCP])
    d_e01, d_e02 = ein("e01", [1, H1 + F]), ein("e02", [1, 1 + F])
    d_dstcol = ein("dstcol", [128, NT], bf16)
    d_dstrow = ein("dstrow", [NBLK, PCMAX], bf16)
    d_idx16 = ein("idx16", [128, IDXCOLS], i16)

    d_out = nc.dram_tensor("outloc", [NLOCP, F], f32, kind="ExternalOutput")

    d_xl2loc = nc.dram_tensor("xl2loc", [NLOCP, F], bf16)
    d_xl2sh = nc.dram_tensor("xl2sh", [NPAD, F], bf16, addr_space="Shared")

    with tile.TileContext(nc) as tc:
        with tc.tile_pool(name="const", bufs=1) as cp:
            Wl1b_sb = cp.tile_from(d_Wl1b[:, :])
            Wr1b_sb = cp.tile_from(d_Wr1b[:, :])
            Wl2_sb = cp.tile_from(d_Wl2[:, :])
            Wr2_sb = cp.tile_from(d_Wr2[:, :])
            att1_sb = cp.tile_from(d_att1[:, :])
            att2_sb = cp.tile_from(d_att2[:, :])
            b1_sb = cp.tile_from(d_b1r[:, :])
            b2_sb = cp.tile_from(d_b2r[:, :])
            id_sb = cp.tile_from(d_id[:, :])
            iorowb_sb = cp.tile_from(d_iorowb[:, :])
            iocolb_sb = cp.tile_from(d_iocolb[:, :])
            padm_sb = cp.tile_from(d_padm[:, :])
            e01_sb = cp.tile_from(d_e01[:, :])
            e02_sb = cp.tile_from(d_e02[:, :])
            dstc_sb = cp.tile_from(d_dstcol[:, :])
            idx_sb = cp.tile_from(d_idx16[:, :])
            xT_sb = cp.tile_from(d_xTloc[:, :])
            xr2_sb = cp.tile([128, NLOCP], bf16)   # layer-2 xr table (SBUF)

            def edge_layer(H, att_sb, e0_sb, bias_sb, get_xl, epilogue,
                           m_eng=None):
                """get_xl(b, wp, pp) -> group(g0, sz) -> (score_in, val_in):
                  score_in: [128, sz*128] e=xl+xr (PSUM or SBUF), val_in:
                  AP covering the block's xl rows (PSUM group or SBUF)."""
                m_eng = m_eng or nc.vector
                with (
                    tc.tile_pool(name="ep", bufs=3) as wp,
                    tc.tile_pool(name="epp", bufs=2, space="PSUM") as pp,
                    tc.tile_pool(name="epp1", bufs=1, space="PSUM") as pp1,
                ):
                    for b in range(NBLK):
                        ntb = tiles_b[b]
                        NTB = ntb * 128
                        t0 = offs[b] // 128
                        # scatter mask m[e, (t, node)]
                        mm = wp.tile([128, PCMAX], bf16, tag="m")
                        m_eng.tensor_tensor(
                            out=mm[:, 0:NTB].rearrange("p (t n) -> p t n",
                                                       t=ntb),
                            in0=dstc_sb[:, t0:t0 + ntb].unsqueeze(2)
                                .to_broadcast([128, ntb, 128]),
                            in1=iorowb_sb[:].unsqueeze(1)
                                .to_broadcast([128, ntb, 128]),
                            op=OP.is_equal)
                        b_acc = pp1.tile([128, H + F], f32, tag="b_acc")
                        nc.tensor.matmul(out=b_acc[:],
                                         lhsT=padm_sb[0:1, b * 128:(b + 1) * 128],
                                         rhs=e0_sb[0:1, 0:H + F],
                                         start=True, stop=False,
                                         skip_group_check=True)
                        src, xr_ap = get_xl(b, wp, pp)
                        for g0 in range(0, ntb, 4):
                            sz = min(4, ntb - g0)
                            E1 = sz * 128
                            pse, val = src(g0, sz)
                            # lrel = prelu(e)  [e, F] group
                            lr = wp.tile([128, 512], bf16, tag="lr")
                            if use_prelu:
                                nc.scalar.activation(
                                    out=lr[:, 0:E1], in_=pse[:, 0:E1],
                                    func=AF.Prelu, alpha=NEG)
                            else:  # CoreSim lacks Prelu
                                relu = wp.tile([128, 512], bf16, tag="relu")
                                nc.scalar.activation(
                                    out=relu[:, 0:E1], in_=pse[:, 0:E1],
                                    func=AF.Relu, scale=1.0 - NEG)
                                nc.vector.scalar_tensor_tensor(
                                    out=lr[:, 0:E1], in0=pse[:, 0:E1],
                                    scalar=NEG, in1=relu[:, 0:E1],
                                    op0=OP.mult, op1=OP.add)
                            # s = reduce(lrel * att)  (flat dense bf16)
                            sm = wp.tile([128, 512], bf16, tag="sm")
                            nc.vector.tensor_tensor(
                                out=sm[:, 0:E1], in0=lr[:, 0:E1],
                                in1=att_sb[:, 0:E1], op=OP.mult)
                            sg = wp.tile([128, 4 * H], f32, tag="sg")
                            nc.vector.reduce_sum(
                                out=sg[:, 0:sz * H]
                                    .rearrange("p (t h) -> p t h", t=sz),
                                in_=sm[:, 0:E1]
                                    .rearrange("p (t h c) -> p t h c",
                                               t=sz, h=H),
                                axis=AX.X)
                            w_sb = wp.tile([128, 4 * (H + F)], bf16,
                                           tag="w_sb")
                            nc.scalar.activation(
                                out=w_sb[:].rearrange("p (t x) -> p t x",
                                                      t=4)[:, 0:sz, 0:H],
                                in_=sg[:, 0:sz * H]
                                    .rearrange("p (t h) -> p t h", t=sz),
                                func=AF.Exp)
                            nc.vector.tensor_tensor(
                                out=w_sb[:].rearrange("p (t x) -> p t x",
                                                      t=4)[:, 0:sz, H:H + F]
                                    .rearrange("p t (h c) -> p t h c", h=H),
                                in0=val.rearrange("p (t h c) -> p t h c",
                                                  t=sz, h=H),
                                in1=w_sb[:].rearrange("p (t x) -> p t x",
                                                      t=4)[:, 0:sz, 0:H]
                                    .unsqueeze(3)
                                    .to_broadcast([128, sz, H, F // H]),
                                op=OP.mult)
                            for j in range(sz):
                                nc.tensor.matmul(
                                    out=b_acc[:],
                                    lhsT=mm[:, (g0 + j) * 128:
                                            (g0 + j + 1) * 128],
                                    rhs=w_sb[:, j * (H + F):
                                             (j + 1) * (H + F)],
                                    start=False,
                                    stop=(g0 + sz >= ntb and j == sz - 1),
                                    skip_group_check=True)
                        # epilogue: divide by denominators, add bias
                        recip = wp.tile([128, H], f32, tag="recip")
                        nc.vector.reciprocal(recip[:], b_acc[:, 0:H])
                        outb = wp.tile([128, F], f32, tag="outb")
                        C = F // H
                        for h in range(H):
                            nc.vector.tensor_scalar_mul(
                                outb[:, h * C:(h + 1) * C],
                                b_acc[:, H + h * C:H + (h + 1) * C],
                                recip[:, h:h + 1])
                        nc.vector.tensor_tensor(out=outb[:], in0=outb[:],
                                                in1=bias_sb[:], op=OP.add)
                        # value path used e = xl + xr; sum(alpha) == 1 per
                        # node, so subtract xr[dst] once to recover sum(
                        # alpha*xl).  Same bf16 xr data path -> cancels
                        # almost exactly.
                        nc.vector.tensor_tensor(out=outb[:], in0=outb[:],
                                                in1=xr_ap, op=OP.subtract)
                        epilogue(b, outb, wp, pp1)

            # ---------------- layer 1 ----------------
            def get_xl_1(b, wp, pp):
                ntb = tiles_b[b]
                NTB = ntb * 128
                o0 = offs[b]
                xe = wp.tile([128, PCMAX], bf16, tag="xe")
                xd = wp.tile([128, PCMAX], bf16, tag="xd")
                nc.sync.dma_start(out=xe[:, 0:NTB],
                                  in_=d_xeT[:, o0:o0 + NTB])
                nc.scalar.dma_start(out=xd[:, 0:NTB],
                                    in_=d_xdT[:, o0:o0 + NTB])

                # per-node xr for the epilogue subtraction (same bf16 path
                # as the xd stream -> the xr in e cancels almost exactly)
                ps_xr = pp.tile([128, 128], f32, tag="psxr")
                nc.tensor.matmul(out=ps_xr[:],
                                 lhsT=xT_sb[:, b * 128:(b + 1) * 128],
                                 rhs=Wr1b_sb[:], start=True, stop=True)

                def group(g0, sz):
                    # pse = e = xl + xr (score AND value path)
                    pse = pp.tile([128, 512], f32, tag="pse")
                    for j in range(sz):
                        sl = slice((g0 + j) * 128, (g0 + j + 1) * 128)
                        ops = slice(j * 128, (j + 1) * 128)
                        nc.tensor.matmul(out=pse[:, ops],
                                         lhsT=xe[:, sl], rhs=Wl1b_sb[:],
                                         start=True, stop=False,
                                         skip_group_check=True)
                        nc.tensor.matmul(out=pse[:, ops],
                                         lhsT=xd[:, sl], rhs=Wr1b_sb[:],
                                         start=False, stop=True,
                                         skip_group_check=True)
                    return pse, pse[:, 0:sz * 128]
                return group, ps_xr

            def epi1(b, outb, wp, pp1):
                ps_h = pp1.tile([128, 128], f32, tag="ps_h")
                nc.tensor.matmul(out=ps_h[:], lhsT=outb[:], rhs=id_sb[:],
                                 is_transpose=True, start=True, stop=True)
                hT = wp.tile([128, 128], f32, tag="hT")
                nc.scalar.copy(hT[:], ps_h[:])
                ps_x = pp1.tile([128, 128], f32, tag="ps_x2")
                nc.tensor.matmul(out=ps_x[:], lhsT=hT[:], rhs=Wr2_sb[:],
                                 start=True, stop=True)
                nc.scalar.copy(xr2_sb[:, b * 128:(b + 1) * 128], ps_x[:])
                ps_l = pp1.tile([128, 128], f32, tag="ps_x2")
                nc.tensor.matmul(out=ps_l[:], lhsT=hT[:], rhs=Wl2_sb[:],
                                 start=True, stop=True)
                l2s = wp.tile([128, 128], bf16, tag="l2s")
                nc.scalar.copy(l2s[:], ps_l[:])
                nc.sync.dma_start(out=d_xl2loc[b * 128:(b + 1) * 128, :],
                                  in_=l2s[:])

            edge_layer(H1, att1_sb, e01_sb, b1_sb, get_xl_1, epi1)

            # ------------- AllGather layer-2 table -------------
            nc.gpsimd.collective_compute(
                "AllGather", mybir.AluOpType.bypass,
                replica_groups=[list(range(NC))],
                ins=[d_xl2loc[:, :]], outs=[d_xl2sh[:, :]],
            )

            # ---------------- layer 2 ----------------
            def get_xl_2(b, wp, pp):
                ntb = tiles_b[b]
                NTB = ntb * 128
                nlo, nhi = NLO_b[b], NHI_b[b]
                c0 = offs[b] // 16
                xlg = wp.tile([128, PCMAX], bf16, tag="xlg")
                if nlo:
                    nc.gpsimd.dma_gather(
                        xlg[:, 0:nlo].rearrange("p (t f) -> p t f", f=F),
                        d_xl2sh[0:SPLIT, :],
                        idx_sb[:, c0:c0 + nlo // 16],
                        nlo, nlo, F, single_packet=False)
                if nhi:
                    nc.gpsimd.dma_gather(
                        xlg[:, nlo:NTB].rearrange("p (t f) -> p t f", f=F),
                        d_xl2sh[SPLIT:NPAD, :],
                        idx_sb[:, c0 + nlo // 16:c0 + NTB // 16],
                        nhi, nhi, F, single_packet=False)
                # m2[node, e] from DMA partition-broadcast of dst row
                drep = wp.tile([128, PCMAX], bf16, tag="drep")
                nc.sync.dma_start(
                    out=drep[:, 0:NTB],
                    in_=d_dstrow[b:b + 1, 0:NTB].to_broadcast([128, NTB]))
                m2 = wp.tile([128, PCMAX], bf16, tag="m2")
                nc.vector.tensor_tensor(
                    out=m2[:, 0:NTB], in0=drep[:, 0:NTB],
                    in1=iocolb_sb[:].to_broadcast([128, NTB]),
                    op=OP.is_equal)

                def group(g0, sz):
                    E1 = sz * 128
                    o1 = g0 * 128
                    psx = pp.tile([128, 512], f32, tag="pse")
                    for j in range(sz):
                        nc.tensor.matmul(
                            out=psx[:, j * 128:(j + 1) * 128],
                            lhsT=m2[:, (g0 + j) * 128:(g0 + j + 1) * 128],
                            rhs=xr2_sb[:, b * 128:(b + 1) * 128],
                            start=True, stop=True, skip_group_check=True)
                    es = wp.tile([128, 512], bf16, tag="es")
                    nc.vector.tensor_tensor(
                        out=es[:, 0:E1], in0=psx[:, 0:E1],
                        in1=xlg[:, o1:o1 + E1], op=OP.add)
                    return es, es[:, 0:E1]
                return group, xr2_sb[:, b * 128:(b + 1) * 128]

            def epi2(b, outb, wp, pp1):
                nc.sync.dma_start(out=d_out[b * 128:(b + 1) * 128, :],
                                  in_=outb[:])

            edge_layer(1, att2_sb, e02_sb, b2_sb, get_xl_2, epi2)

    nc.compile()
    return nc


# --------------------------------------------------------------------------
# Entry point
# --------------------------------------------------------------------------

_NC_CACHE = {}


def kernel(x, edge_index, edge_attr, Wl1, Wr1, att1, b1, Wl2, Wr2, att2, b2,
           cfg=None, _want_results=False, _trace=False):
    from concourse.bass_utils import run_bass_kernel_spmd

    cfg = cfg or CFG
    in_maps, meta = host_prep(x, edge_index, att1, b1, att2, b2, Wl1, Wr1,
                              cfg)
    for m in in_maps:
        m["Wl2"] = np.asarray(Wl2, np.float32)
        m["Wr2"] = np.asarray(Wr2, np.float32)
    key = (cfg.N, cfg.E, tuple(meta["tiles_b"]), tuple(meta["NLO_b"]))
    nc = _NC_CACHE.get(key)
    if nc is None:
        nc = build_nc(cfg, meta)
        _NC_CACHE[key] = nc
    res = run_bass_kernel_spmd(nc, in_maps, core_ids=list(range(cfg.ncores)),
                               trace=_trace)
    out = np.empty((cfg.N, cfg.F), dtype=np.float32)
    for c in range(cfg.ncores):
        out[c * cfg.nloc:(c + 1) * cfg.nloc] = \
            res.results[c]["outloc"][:cfg.nloc]
    if _want_results:
        return out, res
    return out


# revision 49
# speedup vs baseline: 1.0737x; 1.0737x over previous
"""GATv2 (2-layer, N=50000, E=800000) on 8 Trainium2 NeuronCores.

v2 strategy (self-contained; shapes hardcoded for nn_GATUnit_34067680592302):
  - Nodes partitioned across 8 cores (6250 each, 49 blocks of 128). Edges
    (incl. self-loops) assigned by destination block; within a block, edges
    with src < 32768 (padded id) come first, then src >= 32768, so layer-2
    table gathers can use int16-indexed dma_gather on two table halves.
  - Layer 1 needs NO gathers: the host streams per-edge transposed features
    x^T[:, src_e] and x^T[:, dst_e] (bf16), and e = x_src@Wl1 + x_dst@Wr1 is
    accumulated per 128-edge tile directly in PSUM by two matmuls.
  - Scores stay in edge-major layout: s = reduce_sum(prelu(e) * att_bc) on
    the vector engine; exp on scalar; weighted values w = xl_e * exp(s).
  - Scatter-softmax per block via mask matmuls: m[e, node] = (dst_e == node)
    accumulates [exp | w] into PSUM; epilogue divides by the denominators.
  - Layer 2: xl2 = h@Wl2 table is AllGather'ed (bf16) and row-gathered per
    block with two dma_gathers (lo/hi half).  xr2[dst_e] is reconstructed
    without a gather: dstrep = DMA partition-broadcast of the block's dst
    row, m2[node, e] = (dstrep == iota_col), and one matmul per tile expands
    the block-resident xr2 tile onto edges.
"""
import sys
sys.path.insert(0, "/opt/trn_rl_repo")

import numpy as np
import ml_dtypes

NEG = 0.2
BF = ml_dtypes.bfloat16


class Cfg:
    def __init__(self, N=50000, E=800000, ncores=8, nloc=6250, F=128):
        assert N == ncores * nloc
        self.N, self.E, self.ncores, self.nloc, self.F = N, E, ncores, nloc, F
        self.nblk = -(-nloc // 128)          # blocks of 128 nodes per core
        self.nlocp = self.nblk * 128         # padded local nodes
        self.npad = ncores * self.nlocp     # padded global nodes
        self.split = min(32768, self.npad)  # gather-table split (int16)
        assert self.npad % 512 == 0


CFG = Cfg()


def _r128(x):
    return int(-(-x // 128) * 128)


# --------------------------------------------------------------------------
# Host-side preprocessing
# --------------------------------------------------------------------------

def host_prep(x, edge_index, att1, b1, att2, b2, Wl1, Wr1, cfg):
    N, E, NC, NLOC = cfg.N, cfg.E, cfg.ncores, cfg.nloc
    NBLK, NLOCP, NPAD, F = cfg.nblk, cfg.nlocp, cfg.npad, cfg.F
    SPLIT = cfg.split

    src0 = np.asarray(edge_index[0]).astype(np.int64)
    dst0 = np.asarray(edge_index[1]).astype(np.int64)
    loops = np.arange(N, dtype=np.int64)
    SRC = np.concatenate([src0, loops])
    DST = np.concatenate([dst0, loops])
    shard = DST // NLOC
    src_g = ((SRC // NLOC) * NLOCP + (SRC % NLOC)).astype(np.int64)
    dst_loc = (DST - shard * NLOC).astype(np.int64)

    # per (core, block): lo/hi section counts
    per_core = []
    nlo = np.zeros((NC, NBLK), dtype=np.int64)
    nhi = np.zeros((NC, NBLK), dtype=np.int64)
    for c in range(NC):
        sel = np.nonzero(shard == c)[0]
        sg, so, do_, dl = src_g[sel], SRC[sel], DST[sel], dst_loc[sel]
        blk = dl // 128
        islo = sg < SPLIT
        # order: block-major, lo-then-hi within block (stable)
        order = np.lexsort((~islo, blk))
        sg, so, do_, dl, blk, islo = (a[order] for a in
                                      (sg, so, do_, dl, blk, islo))
        for b in range(NBLK):
            m = blk == b
            nlo[c, b] = int((islo & m).sum())
            nhi[c, b] = int((~islo & m).sum())
        per_core.append((sg, so, do_, dl, blk, islo))

    NLO_b = np.array([_r128(nlo[:, b].max()) for b in range(NBLK)])
    NHI_b = np.array([_r128(nhi[:, b].max()) for b in range(NBLK)])
    NTB_b = NLO_b + NHI_b
    offs = np.concatenate([[0], np.cumsum(NTB_b)]).astype(np.int64)
    NTOT = int(offs[-1])
    NT = NTOT // 128
    tiles_b = (NTB_b // 128).astype(np.int64)
    PCMAX = int(NTB_b.max())
    IDXCOLS = NTOT // 16

    H1 = att1.shape[0]
    C1 = att1.shape[1]
    att1bc = np.tile(np.asarray(att1, np.float32).reshape(1, H1 * C1),
                     (128, PCMAX // 128)).astype(BF)
    att2bc = np.tile(np.asarray(att2, np.float32).reshape(1, -1),
                     (128, PCMAX // 128)).astype(BF)

    iota = np.arange(128, dtype=np.float32)
    shared = dict(
        att1bc=att1bc, att2bc=att2bc,
        bias1r=np.tile(np.asarray(b1, np.float32), (128, 1)),
        bias2r=np.tile(np.asarray(b2, np.float32), (128, 1)),
        id128=np.eye(128, dtype=np.float32),
        iorowb=np.tile(iota, (128, 1)).astype(BF),
        iocolb=iota.reshape(128, 1).astype(BF),
        padmat=(np.arange(NLOCP) >= NLOC).astype(np.float32).reshape(1, NLOCP),
        e01=np.concatenate([np.ones(H1, np.float32),
                            np.zeros(F, np.float32)]).reshape(1, H1 + F),
        e02=np.concatenate([np.ones(1, np.float32),
                            np.zeros(F, np.float32)]).reshape(1, 1 + F),
        Wl1b=np.asarray(Wl1, np.float32).astype(BF),
        Wr1b=np.asarray(Wr1, np.float32).astype(BF),
    )

    x = np.asarray(x, dtype=np.float32)
    xbf = x.astype(BF)

    in_maps = []
    for c in range(NC):
        sg, so, do_, dl, blk, islo = per_core[c]
        cblo = np.concatenate([[0], np.cumsum(nlo[c])])
        cbhi = np.concatenate([[0], np.cumsum(nhi[c])])
        srco_arr = np.zeros(NTOT, dtype=np.int64)
        dsto_arr = np.zeros(NTOT, dtype=np.int64)
        dstb_arr = np.full(NTOT, -1.0, dtype=np.float32)
        idx16 = np.zeros(NTOT, dtype=np.int16)
        for b in range(NBLK):
            o = int(offs[b])
            ml = blk == b
            lo_i = np.nonzero(ml & islo)[0]
            hi_i = np.nonzero(ml & ~islo)[0]
            nl, nh = len(lo_i), len(hi_i)
            srco_arr[o:o + nl] = so[lo_i]
            dsto_arr[o:o + nl] = do_[lo_i]
            dstb_arr[o:o + nl] = dl[lo_i] - 128 * b
            idx16[o:o + nl] = sg[lo_i].astype(np.int16)
            o2 = o + int(NLO_b[b])
            srco_arr[o2:o2 + nh] = so[hi_i]
            dsto_arr[o2:o2 + nh] = do_[hi_i]
            dstb_arr[o2:o2 + nh] = dl[hi_i] - 128 * b
            idx16[o2:o2 + nh] = (sg[hi_i] - SPLIT).astype(np.int16)

        dstcol = np.ascontiguousarray(
            dstb_arr.reshape(NT, 128).T).astype(BF)
        dstrow = np.full((NBLK, PCMAX), -1.0, dtype=np.float32)
        for b in range(NBLK):
            dstrow[b, :int(NTB_b[b])] = \
                dstb_arr[int(offs[b]):int(offs[b + 1])]
        dstrow = dstrow.astype(BF)
        idxw = np.ascontiguousarray(idx16.reshape(IDXCOLS, 16).T)
        idxw = np.tile(idxw, (8, 1))

        xeT = np.ascontiguousarray(xbf[srco_arr].T)   # [128, NTOT]
        xdT = np.ascontiguousarray(xbf[dsto_arr].T)   # [128, NTOT]
        xloc = np.zeros((NLOCP, F), dtype=BF)
        xloc[:NLOC] = xbf[c * NLOC:(c + 1) * NLOC]
        xTloc = np.ascontiguousarray(xloc.T)          # [128, NLOCP]

        m = dict(shared)
        m.update(dstcol=dstcol, dstrow=dstrow, idx16=idxw, xeT=xeT, xdT=xdT,
                 xTloc=xTloc)
        in_maps.append(m)

    meta = dict(NLO_b=NLO_b.tolist(), NHI_b=NHI_b.tolist(),
                tiles_b=tiles_b.tolist(), offs=offs.tolist(), NT=NT,
                NTOT=NTOT, PCMAX=PCMAX, IDXCOLS=IDXCOLS, H1=H1)
    return in_maps, meta


# --------------------------------------------------------------------------
# Device program
# --------------------------------------------------------------------------

def build_nc(cfg, meta, use_prelu=True):
    import concourse.bacc as bacc
    import concourse.tile as tile
    from concourse import mybir

    f32 = mybir.dt.float32
    bf16 = mybir.dt.bfloat16
    i16 = mybir.dt.int16
    AF = mybir.ActivationFunctionType
    OP = mybir.AluOpType
    AX = mybir.AxisListType

    NC, F = cfg.ncores, cfg.F
    NBLK, NLOCP, NPAD = cfg.nblk, cfg.nlocp, cfg.npad
    SPLIT = cfg.split
    NT, NTOT, PCMAX = meta["NT"], meta["NTOT"], meta["PCMAX"]
    IDXCOLS, H1 = meta["IDXCOLS"], meta["H1"]
    NLO_b, NHI_b = meta["NLO_b"], meta["NHI_b"]
    tiles_b, offs = meta["tiles_b"], meta["offs"]

    nc = bacc.Bacc("TRN2", target_bir_lowering=False)

    din = {}
    def ein(name, shape, dt=f32):
        din[name] = nc.dram_tensor(name, shape, dt, kind="ExternalInput")
        return din[name]

    d_xeT = ein("xeT", [128, NTOT], bf16)
    d_xdT = ein("xdT", [128, NTOT], bf16)
    d_xTloc = ein("xTloc", [128, NLOCP], bf16)
    d_Wl1b, d_Wr1b = ein("Wl1b", [128, 128], bf16), ein("Wr1b", [128, 128], bf16)
    d_Wl2, d_Wr2 = ein("Wl2", [128, 128]), ein("Wr2", [128, 128])
    d_att1 = ein("att1bc", [128, PCMAX], bf16)
    d_att2 = ein("att2bc", [128, PCMAX], bf16)
    d_b1r, d_b2r = ein("bias1r", [128, F]), ein("bias2r", [128, F])
    d_id = ein("id128", [128, 128])
    d_iorowb = ein("iorowb", [128, 128], bf16)
    d_iocolb = ein("iocolb", [128, 1], bf16)
    d_padm = ein("padmat", [1, NLOCP])
    d_e01, d_e02 = ein("e01", [1, H1 + F]), ein("e02", [1, 1 + F])
    d_dstcol = ein("dstcol", [128, NT], bf16)
    d_dstrow = ein("dstrow", [NBLK, PCMAX], bf16)
    d_idx16 = ein("idx16", [128, IDXCOLS], i16)

    d_out = nc.dram_tensor("outloc", [NLOCP, F], f32, kind="ExternalOutput")

    d_xl2loc = nc.dram_tensor("xl2loc", [NLOCP, F], bf16)
    d_xl2sh = nc.dram_tensor("xl2sh", [NPAD, F], bf16, addr_space="Shared")

    with tile.TileContext(nc) as tc:
        with tc.tile_pool(name="const", bufs=1) as cp:
            Wl1b_sb = cp.tile_from(d_Wl1b[:, :])
            Wr1b_sb = cp.tile_from(d_Wr1b[:, :])
            Wl2_sb = cp.tile_from(d_Wl2[:, :])
            Wr2_sb = cp.tile_from(d_Wr2[:, :])
            att1_sb = cp.tile_from(d_att1[:, :])
            att2_sb = cp.tile_from(d_att2[:, :])
            b1_sb = cp.tile_from(d_b1r[:, :])
            b2_sb = cp.tile_from(d_b2r[:, :])
            id_sb = cp.tile_from(d_id[:, :])
            iorowb_sb = cp.tile_from(d_iorowb[:, :])
            iocolb_sb = cp.tile_from(d_iocolb[:, :])
            padm_sb = cp.tile_from(d_padm[:, :])
            e01_sb = cp.tile_from(d_e01[:, :])
            e02_sb = cp.tile_from(d_e02[:, :])
            dstc_sb = cp.tile_from(d_dstcol[:, :])
            idx_sb = cp.tile_from(d_idx16[:, :])
            xT_sb = cp.tile_from(d_xTloc[:, :])
            xr2_sb = cp.tile([128, NLOCP], bf16)   # layer-2 xr table (SBUF)

            def edge_layer(H, att_sb, e0_sb, bias_sb, get_xl, epilogue,
                           m_eng=None):
                """get_xl(b, wp, pp) -> group(g0, sz) -> (score_in, val_in):
                  score_in: [128, sz*128] e=xl+xr (PSUM or SBUF), val_in:
                  AP covering the block's xl rows (PSUM group or SBUF)."""
                m_eng = m_eng or nc.vector
                with (
                    tc.tile_pool(name="ep", bufs=3) as wp,
                    tc.tile_pool(name="eppe", bufs=3, space="PSUM") as ppe,
                    tc.tile_pool(name="eppx", bufs=2, space="PSUM") as ppx,
                    tc.tile_pool(name="epp1", bufs=1, space="PSUM") as pp1,
                ):
                    for b in range(NBLK):
                        ntb = tiles_b[b]
                        NTB = ntb * 128
                        t0 = offs[b] // 128
                        # scatter mask m[e, (t, node)]
                        mm = wp.tile([128, PCMAX], bf16, tag="m")
                        m_eng.tensor_tensor(
                            out=mm[:, 0:NTB].rearrange("p (t n) -> p t n",
                                                       t=ntb),
                            in0=dstc_sb[:, t0:t0 + ntb].unsqueeze(2)
                                .to_broadcast([128, ntb, 128]),
                            in1=iorowb_sb[:].unsqueeze(1)
                                .to_broadcast([128, ntb, 128]),
                            op=OP.is_equal)
                        b_acc = pp1.tile([128, H + F], f32, tag="b_acc")
                        nc.tensor.matmul(out=b_acc[:],
                                         lhsT=padm_sb[0:1, b * 128:(b + 1) * 128],
                                         rhs=e0_sb[0:1, 0:H + F],
                                         start=True, stop=False,
                                         skip_group_check=True)
                        src, xr_ap = get_xl(b, wp, (ppe, ppx))
                        for g0 in range(0, ntb, 4):
                            sz = min(4, ntb - g0)
                            E1 = sz * 128
                            pse, val = src(g0, sz)
                            # lrel = prelu(e)  [e, F] group
                            lr = wp.tile([128, 512], bf16, tag="lr")
                            if use_prelu:
                                nc.scalar.activation(
                                    out=lr[:, 0:E1], in_=pse[:, 0:E1],
                                    func=AF.Prelu, alpha=NEG)
                            else:  # CoreSim lacks Prelu
                                relu = wp.tile([128, 512], bf16, tag="relu")
                                nc.scalar.activation(
                                    out=relu[:, 0:E1], in_=pse[:, 0:E1],
                                    func=AF.Relu, scale=1.0 - NEG)
                                nc.vector.scalar_tensor_tensor(
                                    out=lr[:, 0:E1], in0=pse[:, 0:E1],
                                    scalar=NEG, in1=relu[:, 0:E1],
                                    op0=OP.mult, op1=OP.add)
                            # s = reduce(lrel * att)  (flat dense bf16)
                            sm = wp.tile([128, 512], bf16, tag="sm")
                            nc.vector.tensor_tensor(
                                out=sm[:, 0:E1], in0=lr[:, 0:E1],
                                in1=att_sb[:, 0:E1], op=OP.mult)
                            sg = wp.tile([128, 4 * H], f32, tag="sg")
                            nc.vector.reduce_sum(
                                out=sg[:, 0:sz * H]
                                    .rearrange("p (t h) -> p t h", t=sz),
                                in_=sm[:, 0:E1]
                                    .rearrange("p (t h c) -> p t h c",
                                               t=sz, h=H),
                                axis=AX.X)
                            w_sb = wp.tile([128, 4 * (H + F)], bf16,
                                           tag="w_sb")
                            nc.scalar.activation(
                                out=w_sb[:].rearrange("p (t x) -> p t x",
                                                      t=4)[:, 0:sz, 0:H],
                                in_=sg[:, 0:sz * H]
                                    .rearrange("p (t h) -> p t h", t=sz),
                                func=AF.Exp)
                            nc.vector.tensor_tensor(
                                out=w_sb[:].rearrange("p (t x) -> p t x",
                                                      t=4)[:, 0:sz, H:H + F]
                                    .rearrange("p t (h c) -> p t h c", h=H),
                                in0=val.rearrange("p (t h c) -> p t h c",
                                                  t=sz, h=H),
                                in1=w_sb[:].rearrange("p (t x) -> p t x",
                                                      t=4)[:, 0:sz, 0:H]
                                    .unsqueeze(3)
                                    .to_broadcast([128, sz, H, F // H]),
                                op=OP.mult)
                            for j in range(sz):
                                nc.tensor.matmul(
                                    out=b_acc[:],
                                    lhsT=mm[:, (g0 + j) * 128:
                                            (g0 + j + 1) * 128],
                                    rhs=w_sb[:, j * (H + F):
                                             (j + 1) * (H + F)],
                                    start=False,
                                    stop=(g0 + sz >= ntb and j == sz - 1),
                                    skip_group_check=True)
                        # epilogue: divide by denominators, add bias
                        recip = wp.tile([128, H], f32, tag="recip")
                        nc.vector.reciprocal(recip[:], b_acc[:, 0:H])
                        outb = wp.tile([128, F], f32, tag="outb")
                        C = F // H
                        for h in range(H):
                            nc.vector.tensor_scalar_mul(
                                outb[:, h * C:(h + 1) * C],
                                b_acc[:, H + h * C:H + (h + 1) * C],
                                recip[:, h:h + 1])
                        nc.vector.tensor_tensor(out=outb[:], in0=outb[:],
                                                in1=bias_sb[:], op=OP.add)
                        # value path used e = xl + xr; sum(alpha) == 1 per
                        # node, so subtract xr[dst] once to recover sum(
                        # alpha*xl).  Same bf16 xr data path -> cancels
                        # almost exactly.
                        nc.vector.tensor_tensor(out=outb[:], in0=outb[:],
                                                in1=xr_ap, op=OP.subtract)
                        epilogue(b, outb, wp, pp1)

            # ---------------- layer 1 ----------------
            def get_xl_1(b, wp, pp):
                ppe, ppx = pp
                ntb = tiles_b[b]
                NTB = ntb * 128
                o0 = offs[b]
                xe = wp.tile([128, PCMAX], bf16, tag="xe")
                xd = wp.tile([128, PCMAX], bf16, tag="xd")
                nc.sync.dma_start(out=xe[:, 0:NTB],
                                  in_=d_xeT[:, o0:o0 + NTB])
                nc.scalar.dma_start(out=xd[:, 0:NTB],
                                    in_=d_xdT[:, o0:o0 + NTB])

                # per-node xr for the epilogue subtraction (same bf16 path
                # as the xd stream -> the xr in e cancels almost exactly)
                ps_xr = ppx.tile([128, 128], f32, tag="psxr")
                nc.tensor.matmul(out=ps_xr[:],
                                 lhsT=xT_sb[:, b * 128:(b + 1) * 128],
                                 rhs=Wr1b_sb[:], start=True, stop=True)

                def group(g0, sz):
                    # pse = e = xl + xr (score AND value path)
                    pse = ppe.tile([128, 512], f32, tag="pse")
                    for j in range(sz):
                        sl = slice((g0 + j) * 128, (g0 + j + 1) * 128)
                        ops = slice(j * 128, (j + 1) * 128)
                        nc.tensor.matmul(out=pse[:, ops],
                                         lhsT=xe[:, sl], rhs=Wl1b_sb[:],
                                         start=True, stop=False,
                                         skip_group_check=True)
                        nc.tensor.matmul(out=pse[:, ops],
                                         lhsT=xd[:, sl], rhs=Wr1b_sb[:],
                                         start=False, stop=True,
                                         skip_group_check=True)
                    return pse, pse[:, 0:sz * 128]
                return group, ps_xr

            def epi1(b, outb, wp, pp1):
                ps_h = pp1.tile([128, 128], f32, tag="ps_h")
                nc.tensor.matmul(out=ps_h[:], lhsT=outb[:], rhs=id_sb[:],
                                 is_transpose=True, start=True, stop=True)
                hT = wp.tile([128, 128], f32, tag="hT")
                nc.scalar.copy(hT[:], ps_h[:])
                ps_x = pp1.tile([128, 128], f32, tag="ps_x2")
                nc.tensor.matmul(out=ps_x[:], lhsT=hT[:], rhs=Wr2_sb[:],
                                 start=True, stop=True)
                nc.scalar.copy(xr2_sb[:, b * 128:(b + 1) * 128], ps_x[:])
                ps_l = pp1.tile([128, 128], f32, tag="ps_x2")
                nc.tensor.matmul(out=ps_l[:], lhsT=hT[:], rhs=Wl2_sb[:],
                                 start=True, stop=True)
                l2s = wp.tile([128, 128], bf16, tag="l2s")
                nc.scalar.copy(l2s[:], ps_l[:])
                nc.sync.dma_start(out=d_xl2loc[b * 128:(b + 1) * 128, :],
                                  in_=l2s[:])

            edge_layer(H1, att1_sb, e01_sb, b1_sb, get_xl_1, epi1)

            # ------------- AllGather layer-2 table -------------
            nc.gpsimd.collective_compute(
                "AllGather", mybir.AluOpType.bypass,
                replica_groups=[list(range(NC))],
                ins=[d_xl2loc[:, :]], outs=[d_xl2sh[:, :]],
            )

            # ---------------- layer 2 ----------------
            def get_xl_2(b, wp, pp):
                ppe, ppx = pp
                ntb = tiles_b[b]
                NTB = ntb * 128
                nlo, nhi = NLO_b[b], NHI_b[b]
                c0 = offs[b] // 16
                xlg = wp.tile([128, PCMAX], bf16, tag="xlg")
                if nlo:
                    nc.gpsimd.dma_gather(
                        xlg[:, 0:nlo].rearrange("p (t f) -> p t f", f=F),
                        d_xl2sh[0:SPLIT, :],
                        idx_sb[:, c0:c0 + nlo // 16],
                        nlo, nlo, F, single_packet=False)
                if nhi:
                    nc.gpsimd.dma_gather(
                        xlg[:, nlo:NTB].rearrange("p (t f) -> p t f", f=F),
                        d_xl2sh[SPLIT:NPAD, :],
                        idx_sb[:, c0 + nlo // 16:c0 + NTB // 16],
                        nhi, nhi, F, single_packet=False)
                # m2[node, e] from DMA partition-broadcast of dst row
                drep = wp.tile([128, PCMAX], bf16, tag="drep")
                nc.sync.dma_start(
                    out=drep[:, 0:NTB],
                    in_=d_dstrow[b:b + 1, 0:NTB].to_broadcast([128, NTB]))
                m2 = wp.tile([128, PCMAX], bf16, tag="m2")
                nc.vector.tensor_tensor(
                    out=m2[:, 0:NTB], in0=drep[:, 0:NTB],
                    in1=iocolb_sb[:].to_broadcast([128, NTB]),
                    op=OP.is_equal)

                def group(g0, sz):
                    E1 = sz * 128
                    o1 = g0 * 128
                    psx = ppe.tile([128, 512], f32, tag="pse")
                    for j in range(sz):
                        nc.tensor.matmul(
                            out=psx[:, j * 128:(j + 1) * 128],
                            lhsT=m2[:, (g0 + j) * 128:(g0 + j + 1) * 128],
                            rhs=xr2_sb[:, b * 128:(b + 1) * 128],
                            start=True, stop=True, skip_group_check=True)
                    es = wp.tile([128, 512], bf16, tag="es")
                    nc.vector.tensor_tensor(
                        out=es[:, 0:E1], in0=psx[:, 0:E1],
                        in1=xlg[:, o1:o1 + E1], op=OP.add)
                    return es, es[:, 0:E1]
                return group, xr2_sb[:, b * 128:(b + 1) * 128]

            def epi2(b, outb, wp, pp1):
                nc.sync.dma_start(out=d_out[b * 128:(b + 1) * 128, :],
                                  in_=outb[:])

            edge_layer(1, att2_sb, e02_sb, b2_sb, get_xl_2, epi2)

    nc.compile()
    return nc


# --------------------------------------------------------------------------
# Entry point
# --------------------------------------------------------------------------

_NC_CACHE = {}


def kernel(x, edge_index, edge_attr, Wl1, Wr1, att1, b1, Wl2, Wr2, att2, b2,
           cfg=None, _want_results=False, _trace=False):
    from concourse.bass_utils import run_bass_kernel_spmd

    cfg = cfg or CFG
    in_maps, meta = host_prep(x, edge_index, att1, b1, att2, b2, Wl1, Wr1,
                              cfg)
    for m in in_maps:
        m["Wl2"] = np.asarray(Wl2, np.float32)
        m["Wr2"] = np.asarray(Wr2, np.float32)
    key = (cfg.N, cfg.E, tuple(meta["tiles_b"]), tuple(meta["NLO_b"]))
    nc = _NC_CACHE.get(key)
    if nc is None:
        nc = build_nc(cfg, meta)
        _NC_CACHE[key] = nc
    res = run_bass_kernel_spmd(nc, in_maps, core_ids=list(range(cfg.ncores)),
                               trace=_trace)
    out = np.empty((cfg.N, cfg.F), dtype=np.float32)
    for c in range(cfg.ncores):
        out[c * cfg.nloc:(c + 1) * cfg.nloc] = \
            res.results[c]["outloc"][:cfg.nloc]
    if _want_results:
        return out, res
    return out
